# revision 17
# baseline (speedup 1.0000x reference)
"""GeomGCN (2-layer relational GCN) distributed Bass kernel for 8 TRN2 NeuronCores.

v2 strategy (node-sharded, graph-parallel, bf16 datapath):
  - Nodes split into 8 slices; core k owns slice k (both as source and dest).
  - Within each slice, dest nodes are permuted host-side (degree-sorted snake
    over TPS tiles of 128) to flatten per-(core, dest-tile) edge counts, which
    minimizes the number of 128-edge chunks (and hence gather descriptors,
    one-hot builds and segment matmuls).
  - Layer math: y1[r,src] = dinv[src]*(x[src] @ W1_r)  (dense bf16 matmuls),
    stored as a DRAM table with interleaved rows (src*R + r), 256B each.
    Edges (sorted by dest, chunked 128/dest-tile) gather their rows with
    dma_gather; a one-hot matrix (built on DVE via broadcast is_equal against
    an iota) turns the per-tile segment-sum into TensorE matmuls in PSUM.
    Pad slots carry a sentinel dest (200) so their one-hot column is all-zero:
    no dummy table rows needed.  Per-node partials are ReduceScattered (bf16);
    layer 2 repeats with 16-wide messages stored in the low 16 columns of
    another interleaved 256B-row table (same gather indices), then a fused
    full-width log_softmax.
  - All DMAs are batched (staged writes of 8-16 tiles, single table loads,
    resident index tables) to amortize the ~650ns fixed per-DMA dispatch cost.
"""
import math
import os
import numpy as np

import concourse.bass as bass
import concourse.tile as tile
from concourse import bacc, mybir
from concourse.bass_utils import run_bass_kernel_spmd
from concourse.masks import make_identity

F32 = mybir.dt.float32
BF16 = mybir.dt.bfloat16
I16 = mybir.dt.int16
BF_NP = mybir.dt.np(mybir.dt.bfloat16)
AF = mybir.ActivationFunctionType
ALU = mybir.AluOpType


class Cfg:
    def __init__(self, N, E, F, H, C, R, ncores=8, B=8, J=8):
        self.N, self.E, self.F, self.H, self.C, self.R = N, E, F, H, C, R
        self.ncores = ncores
        self.NSL = math.ceil(N / ncores)             # real nodes per slice
        tps_nodes = math.ceil(self.NSL / 128)
        tps_edges = math.ceil(E / (ncores * ncores) / 224)
        self.TPS = max(tps_nodes, tps_edges)         # dest tiles per slice
        self.NLOC = self.TPS * 128                   # padded nodes per slice
        self.MC = self.TPS                           # m-chunks per slice
        self.N_PAD = ncores * self.NLOC
        self.NT = ncores * self.TPS                  # global dest tiles
        self.KC = F // 128
        self.B = B                                   # gather chunks per batch
        self.J = J                                   # chunks per one-hot build
        self.GT1 = 8                                 # agg1 tiles per staged DMA
        self.GT2 = 16                                # agg2 tiles per staged DMA
        assert F % 128 == 0 and H == 128
        assert R * self.NLOC < 32768, "int16 gather index overflow"
        assert self.NT % self.GT1 == 0 and self.NT % self.GT2 == 0
        assert math.ceil(self.NSL / self.TPS) <= 128


CFG = Cfg(N=50000, E=800000, F=256, H=128, C=16, R=4)


# ----------------------------------------------------------------- host side
def preprocess(cfg, x, edge_index, edge_relation, W1, b1, W2, b2):
    N, nc8 = cfg.N, cfg.ncores
    NSL, NLOC, TPS, NT, MC, R, B = (cfg.NSL, cfg.NLOC, cfg.TPS, cfg.NT,
                                    cfg.MC, cfg.R, cfg.B)
    row = np.asarray(edge_index[0], dtype=np.int64)
    col = np.asarray(edge_index[1], dtype=np.int64)
    rel = np.asarray(edge_relation, dtype=np.int64)
    x = np.asarray(x, dtype=np.float32)

    deg = np.bincount(row, minlength=N).astype(np.float32)

    # per-slice balancing permutation: degree-sorted snake over TPS tiles
    newloc = np.empty(N, dtype=np.int64)
    for j in range(nc8):
        lo = j * NSL
        hi = min(N, lo + NSL)
        n = hi - lo
        order = np.argsort(-deg[lo:hi], kind="stable")
        rr = np.arange(n)
        rnd, idx = rr // TPS, rr % TPS
        tile_i = np.where(rnd % 2 == 0, idx, TPS - 1 - idx)
        pos = tile_i * 128 + rnd
        nl = np.empty(n, dtype=np.int64)
        nl[order] = pos
        newloc[lo:hi] = nl
    cfg.newloc = newloc

    er = np.minimum(row // NSL, nc8 - 1) * NLOC + newloc[row]  # new dest id
    ksrc = np.minimum(col // NSL, nc8 - 1)
    ecl = newloc[col]                                          # new src local

    # one global sort by (src core, dest id) replaces 8 masked argsorts
    o = np.argsort((ksrc * (nc8 * NLOC) + er).astype(np.int32),
                   kind="stable")
    ers, ecs, egs = er[o], ecl[o], rel[o]
    kofs = np.concatenate([[0], np.cumsum(np.bincount(ksrc, minlength=nc8))])
    counts = np.zeros((nc8, NT), dtype=np.int64)
    percore = []
    for k in range(nc8):
        sl = slice(kofs[k], kofs[k + 1])
        erk, eck, egk = ers[sl], ecs[sl], egs[sl]
        t = erk >> 7
        counts[k] = np.bincount(t, minlength=NT)
        percore.append((erk, eck, egk, t))

    chunks_t = np.maximum(1, np.ceil(counts.max(axis=0) / 128).astype(np.int64))
    CH = int(chunks_t.sum())
    CHpad = math.ceil(CH / B) * B
    NB = CHpad // B
    slot_base = np.concatenate([[0], np.cumsum(chunks_t * 128)])[:-1]

    x16 = x.astype(BF_NP)
    W1b = np.asarray(W1, dtype=np.float32).astype(BF_NP)
    W2cat = (np.asarray(W2, dtype=np.float32)
             .reshape(R, cfg.H, cfg.C).transpose(1, 0, 2)
             .reshape(cfg.H, R * cfg.C).astype(BF_NP))
    b1c = np.asarray(b1, dtype=np.float32).reshape(cfg.H, 1)
    b2r = np.broadcast_to(np.asarray(b2, dtype=np.float32),
                          (128, cfg.C)).copy()

    def core_inputs(k):
        erk, eck, egk, t = percore[k]
        first = np.searchsorted(t, np.arange(NT), side="left")
        rank = np.arange(len(t)) - first[t]
        slots = slot_base[t] + rank
        gidx = np.zeros(CHpad * 128, dtype=np.int16)
        gidx[slots] = (eck * R + egk).astype(np.int16)
        dloc = np.full(CHpad * 128, 200.0, dtype=np.float32)
        dloc[slots] = (erk % 128).astype(np.float32)

        # wrapped-16 index layout, compact (replicated to 128 on device):
        # slot i of batch b -> partition i%16, free column i//16
        gw = np.ascontiguousarray(
            gidx.reshape(NB, B * 8, 16).transpose(2, 0, 1)  # [16, NB, B*8]
        ).reshape(16, NB * B * 8)
        dloc_w = np.ascontiguousarray(
            dloc.reshape(CHpad, 128).T).astype(np.uint8)     # [128, CHpad]

        lo = k * NSL
        hi = min(N, lo + NSL)
        xk = np.zeros((NLOC, cfg.F), dtype=BF_NP)
        xk[newloc[lo:hi]] = x16[lo:hi]
        xTb = np.ascontiguousarray(xk.view(np.uint16).T).view(BF_NP)
        dk = np.zeros(NLOC, dtype=np.float32)
        dk[newloc[lo:hi]] = deg[lo:hi]
        degc = np.ascontiguousarray(dk.reshape(MC, 128).T)   # [128, MC]

        return {
            "xT": xTb,
            "degc": degc,
            "W1": W1b,
            "W2c": W2cat,
            "b1c": b1c,
            "b2r": b2r,
            "gidx": gw,
            "dloc": dloc_w,
        }

    from concurrent.futures import ThreadPoolExecutor
    with ThreadPoolExecutor(max_workers=nc8) as ex:
        in_maps = list(ex.map(core_inputs, range(nc8)))
    return in_maps, tuple(int(v) for v in chunks_t), CHpad


def assemble(cfg, outs):
    """Un-permute per-core outputs into the full [N, C] array."""
    full = np.empty((cfg.N, cfg.C), dtype=np.float32)
    for j in range(cfg.ncores):
        lo = j * cfg.NSL
        hi = min(cfg.N, lo + cfg.NSL)
        full[lo:hi] = outs[j][cfg.newloc[lo:hi]].astype(np.float32)
    return full


# --------------------------------------------------------------- device side
def build_program(cfg, chunks_t, CHpad):
    R, H, C, F = cfg.R, cfg.H, cfg.C, cfg.F
    NB = CHpad // cfg.B
    nc = bacc.Bacc("TRN2", target_bir_lowering=False, debug=False,
                   num_devices=cfg.ncores)

    xT = nc.dram_tensor("xT", [F, cfg.NLOC], BF16, kind="ExternalInput").ap()
    degc = nc.dram_tensor("degc", [128, cfg.MC], F32, kind="ExternalInput").ap()
    W1 = nc.dram_tensor("W1", [R * F, H], BF16, kind="ExternalInput").ap()
    W2c = nc.dram_tensor("W2c", [H, R * C], BF16, kind="ExternalInput").ap()
    b1c = nc.dram_tensor("b1c", [H, 1], F32, kind="ExternalInput").ap()
    b2r = nc.dram_tensor("b2r", [128, C], F32, kind="ExternalInput").ap()
    gidx = nc.dram_tensor("gidx", [16, NB * cfg.B * 8], I16,
                          kind="ExternalInput").ap()
    dloc = nc.dram_tensor("dloc", [128, CHpad], mybir.dt.uint8,
                          kind="ExternalInput").ap()
    out = nc.dram_tensor("out", [cfg.NLOC, C], BF16,
                         kind="ExternalOutput").ap()

    with tile.TileContext(nc) as tc:
        _build(tc, cfg, chunks_t, CHpad, xT, degc, W1, W2c, b1c, b2r,
               gidx, dloc, out)
    nc.compile()
    return nc


def _build(tc, cfg, chunks_t, CHpad, xT, degc, W1, W2c, b1c, b2r,
           gidx, dloc, out):
    nc = tc.nc
    R, H, C = cfg.R, cfg.H, cfg.C
    B, J, MC, NT, KC = cfg.B, cfg.J, cfg.MC, cfg.NT, cfg.KC
    NB = CHpad // B
    RC = R * C
    B8 = B * 8
    with tc.tile_pool(name="const", bufs=1) as cpool, \
         tc.tile_pool(name="big", bufs=1) as bigp, \
         tc.tile_pool(name="gY", bufs=6) as gpool, \
         tc.tile_pool(name="s3", bufs=4) as spool, \
         tc.tile_pool(name="stage", bufs=6) as stpool, \
         tc.tile_pool(name="psum", bufs=6, space="PSUM") as pp, \
         tc.tile_pool(name="dram", bufs=1, space="DRAM") as dram:

        # ---------- constants
        iota16 = cpool.tile([128, 128], I16)
        nc.gpsimd.iota(iota16[:], pattern=[[1, 128]], base=0,
                       channel_multiplier=0)
        iotab = cpool.tile([128, 1, 128], BF16)
        nc.vector.tensor_copy(out=iotab[:, 0, :], in_=iota16[:])
        identf = cpool.tile([128, 128], F32)
        make_identity(nc, identf[:])
        identb = cpool.tile([128, 128], BF16)
        nc.vector.tensor_copy(out=identb[:], in_=identf[:])
        b2t = cpool.tile([128, C], F32)
        nc.sync.dma_start(out=b2t[:], in_=b2r[:, :])
        b1t = cpool.tile([H, 1], F32)
        nc.sync.dma_start(out=b1t[:], in_=b1c[:, :])
        b1tb = cpool.tile([H, 1], BF16)
        nc.vector.tensor_copy(out=b1tb[:], in_=b1t[:])
        w2t = cpool.tile([H, RC], BF16)
        nc.sync.dma_start(out=w2t[:], in_=W2c[:, :])
        w1t = cpool.tile([128, R * KC, H], BF16)
        nc.sync.dma_start(out=w1t[:],
                          in_=W1.rearrange("(q p) h -> p q h", p=128))

        degt = cpool.tile([128, MC], F32)
        nc.sync.dma_start(out=degt[:], in_=degc[:, :])
        dmask = cpool.tile([128, MC], F32)
        nc.vector.tensor_scalar(out=dmask[:], in0=degt[:], scalar1=0.0,
                                scalar2=None, op0=ALU.is_gt)
        dsq = cpool.tile([128, MC], F32)
        nc.scalar.sqrt(out=dsq[:], in_=degt[:])
        drcp = cpool.tile([128, MC], F32)
        nc.vector.reciprocal(out=drcp[:], in_=dsq[:])
        dinv = cpool.tile([128, MC], F32)
        nc.vector.tensor_mul(out=dinv[:], in0=drcp[:], in1=dmask[:])
        dinv2 = cpool.tile([128, MC], F32)
        nc.vector.tensor_mul(out=dinv2[:], in0=dinv[:], in1=dinv[:])

        # resident gather-index table, replicated 16 -> 128 partitions
        idxt = bigp.tile([128, NB * B8], I16)
        nc.sync.dma_start(out=idxt[0:16, :], in_=gidx[:, :])
        nc.sync.dma_start(out=idxt[16:32, :], in_=idxt[0:16, :])
        nc.sync.dma_start(out=idxt[32:64, :], in_=idxt[0:32, :])
        nc.sync.dma_start(out=idxt[64:128, :], in_=idxt[0:64, :])
        dloc8 = bigp.tile([128, CHpad], mybir.dt.uint8)
        nc.sync.dma_start(out=dloc8[:], in_=dloc[:, :])
        dloct = bigp.tile([128, CHpad, 1], BF16)
        nc.vector.tensor_copy(out=dloct[:, :, 0], in_=dloc8[:])

        # crow[r*C+c] = b1 @ W2_r, replicated to 128 partitions
        psc = pp.tile([1, RC], F32, tag="ps")
        nc.tensor.matmul(out=psc[:], lhsT=b1tb[:], rhs=w2t[:],
                         start=True, stop=True)
        crow1 = cpool.tile([1, RC], BF16)
        nc.scalar.copy(out=crow1[:], in_=psc[:])
        onesb = cpool.tile([1, 128], BF16)
        nc.vector.memset(onesb[:], 1.0)
        pscb = pp.tile([128, RC], F32, tag="ps")
        nc.tensor.matmul(out=pscb[:], lhsT=onesb[:], rhs=crow1[:],
                         start=True, stop=True)
        crow128 = cpool.tile([128, RC], F32)
        nc.scalar.copy(out=crow128[:], in_=pscb[:])

        # ---------- dense layer 1: y1[(m*R+r) row] = dinv[m]*(x[m] @ W1_r)
        uT = bigp.tile([128, KC, cfg.NLOC], BF16)
        nc.sync.dma_start(out=uT[:],
                          in_=xT.rearrange("(c p) n -> p c n", p=128))
        y1s = bigp.tile([128, MC, R, H], BF16)
        for mc in range(MC):
            for r in range(R):
                ps = pp.tile([128, H], F32, tag="ps")
                for kc in range(KC):
                    nc.tensor.matmul(
                        out=ps[:],
                        lhsT=uT[:, kc, mc * 128:(mc + 1) * 128],
                        rhs=w1t[:, r * KC + kc, :],
                        start=(kc == 0), stop=(kc == KC - 1))
                if (mc * R + r) % 2 == 0:
                    nc.scalar.mul(out=y1s[:, mc, r, :], in_=ps[:],
                                  mul=dinv[:, mc:mc + 1])
                else:
                    nc.vector.tensor_scalar(out=y1s[:, mc, r, :], in0=ps[:],
                                            scalar1=dinv[:, mc:mc + 1],
                                            scalar2=None, op0=ALU.mult)
        y1d = dram.tile([MC * 128 * R, H], BF16)
        nc.sync.dma_start(
            out=y1d.rearrange("(m p r) h -> p m r h", p=128, r=R),
            in_=y1s[:])

        LIMIT = int(os.environ.get("KLIMIT", "6"))
        if LIMIT < 2:
            return

        def agg_pass(table_ap, width, part_dram, GT, evac):
            """Gather + one-hot matmul segment sum; staged group writes."""
            partv = part_dram.rearrange("(t p) w -> p t w", p=128)
            c = 0
            s3 = None
            g = None
            stage = None
            for t in range(NT):
                if t % GT == 0:
                    stage = stpool.tile([128, GT, width], BF16,
                                        tag=f"stg{GT}_{width}")
                ps = pp.tile([128, width], F32, tag="ps")
                for j in range(chunks_t[t]):
                    if c % B == 0:
                        b = c // B
                        g = gpool.tile([128, B, 128], BF16, tag="g")
                        nc.gpsimd.dma_gather(
                            out_ap=g[:], in_ap=table_ap,
                            idxs_ap=idxt[:, b * B8:(b + 1) * B8],
                            num_idxs=B * 128, num_idxs_reg=B * 128,
                            elem_size=128)
                    if c % J == 0:
                        s3 = spool.tile([128, J, 128], BF16, tag="s3")
                        nj = min(J, CHpad - c)
                        nc.vector.tensor_tensor(
                            out=s3[:, :nj, :],
                            in0=dloct[:, c:c + nj, :].to_broadcast(
                                [128, nj, 128]),
                            in1=iotab[:].to_broadcast([128, nj, 128]),
                            op=ALU.is_equal)
                    nc.tensor.matmul(
                        out=ps[:], lhsT=s3[:, c % J, :],
                        rhs=g[:, c % B, :width],
                        start=(j == 0), stop=(j == chunks_t[t] - 1))
                    c += 1
                evac(stage[:, t % GT, :], ps)
                if t % GT == GT - 1:
                    t0 = t - GT + 1
                    nc.sync.dma_start(out=partv[:, t0:t0 + GT, :],
                                      in_=stage[:])

        def evac_dve(dst, ps):
            nc.vector.tensor_copy(out=dst, in_=ps[:])

        def evac_act(dst, ps):
            nc.scalar.copy(out=dst, in_=ps[:])

        # ---------- layer-1 aggregation + reduce-scatter (bf16)
        t1p = dram.tile([cfg.N_PAD, H], BF16)
        t1r = dram.tile([cfg.NLOC, H], BF16)
        agg_pass(y1d[:], H, t1p, cfg.GT1, evac_act)
        if LIMIT < 3:
            return
        nc.gpsimd.collective_compute(
            "ReduceScatter", ALU.add,
            replica_groups=[list(range(cfg.ncores))],
            ins=[t1p.opt()], outs=[t1r.opt()])
        if LIMIT < 4:
            return

        # ---------- layer-2 dense: y2 rows (m*R+r), cols 0:C used
        t1rs = bigp.tile([128, MC, H], BF16)
        nc.sync.dma_start(out=t1rs[:],
                          in_=t1r.rearrange("(m p) h -> p m h", p=128))
        u2T = bigp.tile([128, cfg.NLOC], BF16)
        for mc in range(MC):
            tt = stpool.tile([128, H], BF16, tag="tt")
            nc.vector.tensor_scalar(out=tt[:], in0=t1rs[:, mc, :],
                                    scalar1=dinv2[:, mc:mc + 1],
                                    scalar2=None, op0=ALU.mult)
            pst = pp.tile([128, 128], BF16, tag="ps")
            nc.tensor.transpose(out=pst[:], in_=tt[:], identity=identb[:])
            nc.vector.tensor_copy(out=u2T[:, mc * 128:(mc + 1) * 128],
                                  in_=pst[:])
        y2s = bigp.tile([128, MC, RC], BF16)
        for mc in range(MC):
            ps2 = pp.tile([128, RC], F32, tag="ps")
            nc.tensor.matmul(out=ps2[:],
                             lhsT=u2T[:, mc * 128:(mc + 1) * 128],
                             rhs=w2t[:], start=True, stop=True)
            bias = stpool.tile([128, RC], F32, tag="bias")
            nc.vector.tensor_scalar(out=bias[:], in0=crow128[:],
                                    scalar1=dinv[:, mc:mc + 1],
                                    scalar2=None, op0=ALU.mult)
            nc.vector.tensor_tensor(out=y2s[:, mc, :], in0=ps2[:],
                                    in1=bias[:], op=ALU.add)
        y2d = dram.tile([MC * 128 * R, 128], BF16)
        y2dv = y2d.rearrange("(m p r) h -> p m r h", p=128, r=R)
        for r in range(R):
            nc.sync.dma_start(out=y2dv[:, :, r, 0:C],
                              in_=y2s[:, :, r * C:(r + 1) * C])
        if LIMIT < 5:
            return

        # ---------- layer-2 aggregation + reduce-scatter
        t2p = dram.tile([cfg.N_PAD, C], BF16)
        t2r = dram.tile([cfg.NLOC, C], BF16)
        agg_pass(y2d[:], C, t2p, cfg.GT2, evac_act)
        nc.gpsimd.collective_compute(
            "ReduceScatter", ALU.add,
            replica_groups=[list(range(cfg.ncores))],
            ins=[t2p.opt()], outs=[t2r.opt()])
        if LIMIT < 6:
            return

        # ---------- final: h2 = dinv*t2 + b2 ; fused log_softmax
        t2s = bigp.tile([128, MC, C], BF16)
        nc.sync.dma_start(out=t2s[:],
                          in_=t2r.rearrange("(m p) c -> p m c", p=128))
        ft = bigp.tile([128, MC, C], F32)
        nc.vector.tensor_tensor(
            out=ft[:], in0=t2s[:],
            in1=dinv[:].unsqueeze(2).to_broadcast([128, MC, C]), op=ALU.mult)
        nc.vector.tensor_tensor(
            out=ft[:], in0=ft[:],
            in1=b2t[:].unsqueeze(1).to_broadcast([128, MC, C]), op=ALU.add)
        negmx = bigp.tile([128, MC], F32)
        nc.vector.tensor_reduce(out=negmx[:], in_=ft[:],
                                axis=mybir.AxisListType.X,
                                op=ALU.max, negate=True)
        nc.vector.tensor_tensor(
            out=ft[:], in0=ft[:],
            in1=negmx[:].unsqueeze(2).to_broadcast([128, MC, C]), op=ALU.add)
        ex = bigp.tile([128, MC, C], F32)
        nc.scalar.activation(out=ex[:], in_=ft[:], func=AF.Exp)
        ssum = bigp.tile([128, MC], F32)
        nc.vector.tensor_reduce(out=ssum[:], in_=ex[:],
                                axis=mybir.AxisListType.X, op=ALU.add)
        lg = bigp.tile([128, MC], F32)
        nc.scalar.activation(out=lg[:], in_=ssum[:], func=AF.Ln)
        fb = bigp.tile([128, MC, C], BF16)
        nc.vector.tensor_tensor(
            out=fb[:], in0=ft[:],
            in1=lg[:].unsqueeze(2).to_broadcast([128, MC, C]),
            op=ALU.subtract)
        nc.sync.dma_start(out=out.rearrange("(m p) c -> p m c", p=128),
                          in_=fb[:])


# ------------------------------------------------------------------ runtime
_PROGRAM_CACHE = {}


def run(cfg, inputs):
    in_maps, chunks_t, CHpad = preprocess(cfg, **inputs)
    key = (cfg.N, cfg.E, chunks_t, CHpad)
    if key not in _PROGRAM_CACHE:
        _PROGRAM_CACHE[key] = build_program(cfg, chunks_t, CHpad)
    nc = _PROGRAM_CACHE[key]
    res = None
    for attempt in range(3):
        try:
            res = run_bass_kernel_spmd(nc, in_maps,
                                       core_ids=list(range(cfg.ncores)))
            break
        except Exception:
            if attempt == 2:
                raise
    outs = [np.asarray(res.results[k]["out"]) for k in range(cfg.ncores)]
    return np.ascontiguousarray(assemble(cfg, outs).astype(np.float32))


def kernel(x, edge_index, edge_relation, W1, b1, W2, b2):
    return run(CFG, dict(x=x, edge_index=edge_index,
                         edge_relation=edge_relation,
                         W1=W1, b1=b1, W2=W2, b2=b2))


# revision 21
# speedup vs baseline: 1.2560x; 1.2560x over previous
"""GeomGCN (2-layer relational GCN) distributed Bass kernel for 8 TRN2 NeuronCores.

v2 strategy (node-sharded, graph-parallel, bf16 datapath):
  - Nodes split into 8 slices; core k owns slice k (both as source and dest).
  - Within each slice, dest nodes are permuted host-side (degree-sorted snake
    over TPS tiles of 128) to flatten per-(core, dest-tile) edge counts, which
    minimizes the number of 128-edge chunks (and hence gather descriptors,
    one-hot builds and segment matmuls).
  - Layer math: y1[r,src] = dinv[src]*(x[src] @ W1_r)  (dense bf16 matmuls),
    stored as a DRAM table with interleaved rows (src*R + r), 256B each.
    Edges (sorted by dest, chunked 128/dest-tile) gather their rows with
    dma_gather; a one-hot matrix (built on DVE via broadcast is_equal against
    an iota) turns the per-tile segment-sum into TensorE matmuls in PSUM.
    Pad slots carry a sentinel dest (200) so their one-hot column is all-zero:
    no dummy table rows needed.  Per-node partials are ReduceScattered (bf16);
    layer 2 repeats with 16-wide messages stored in the low 16 columns of
    another interleaved 256B-row table (same gather indices), then a fused
    full-width log_softmax.
  - All DMAs are batched (staged writes of 8-16 tiles, single table loads,
    resident index tables) to amortize the ~650ns fixed per-DMA dispatch cost.
"""
import math
import os
import numpy as np

import concourse.bass as bass
import concourse.tile as tile
from concourse import bacc, mybir
from concourse.bass_utils import run_bass_kernel_spmd
from concourse.masks import make_identity

F32 = mybir.dt.float32
BF16 = mybir.dt.bfloat16
I16 = mybir.dt.int16
BF_NP = mybir.dt.np(mybir.dt.bfloat16)
AF = mybir.ActivationFunctionType
ALU = mybir.AluOpType


class Cfg:
    def __init__(self, N, E, F, H, C, R, ncores=8, B=8, J=8):
        self.N, self.E, self.F, self.H, self.C, self.R = N, E, F, H, C, R
        self.ncores = ncores
        self.NSL = math.ceil(N / ncores)             # real nodes per slice
        tps_nodes = math.ceil(self.NSL / 128)
        tps_edges = math.ceil(E / (ncores * ncores) / 224)
        self.TPS = max(tps_nodes, tps_edges)         # dest tiles per slice
        self.NLOC = self.TPS * 128                   # padded nodes per slice
        self.MC = self.TPS                           # m-chunks per slice
        self.N_PAD = ncores * self.NLOC
        self.NT = ncores * self.TPS                  # global dest tiles
        self.KC = F // 128
        self.B = B                                   # gather chunks per batch
        self.J = J                                   # chunks per one-hot build
        if self.TPS % 2:                             # halves for split RS
            self.TPS += 1
            self.NLOC = self.TPS * 128
            self.MC = self.TPS
            self.N_PAD = ncores * self.NLOC
            self.NT = ncores * self.TPS
        self.GT1 = min(8, self.NT // 2)              # agg1 tiles per staged DMA
        self.GT2 = min(16, self.NT // 2)             # agg2 tiles per staged DMA
        assert F % 128 == 0 and H == 128
        assert R * self.NLOC < 32768, "int16 gather index overflow"
        assert (self.NT // 2) % self.GT1 == 0 and (self.NT // 2) % self.GT2 == 0
        assert math.ceil(self.NSL / self.TPS) <= 128
        # half-major processing order: all slices' low tiles, then high tiles
        TPS, NTh = self.TPS, self.NT // 2
        self.torder = [s * TPS + m + h * TPS // 2
                       for h in range(2)
                       for s in range(ncores)
                       for m in range(TPS // 2)]


CFG = Cfg(N=50000, E=800000, F=256, H=128, C=16, R=4)


# ----------------------------------------------------------------- host side
def preprocess(cfg, x, edge_index, edge_relation, W1, b1, W2, b2):
    N, nc8 = cfg.N, cfg.ncores
    NSL, NLOC, TPS, NT, MC, R, B = (cfg.NSL, cfg.NLOC, cfg.TPS, cfg.NT,
                                    cfg.MC, cfg.R, cfg.B)
    row = np.asarray(edge_index[0], dtype=np.int64)
    col = np.asarray(edge_index[1], dtype=np.int64)
    rel = np.asarray(edge_relation, dtype=np.int64)
    x = np.asarray(x, dtype=np.float32)

    deg = np.bincount(row, minlength=N).astype(np.float32)

    # per-slice balancing permutation: degree-sorted snake over TPS tiles
    newloc = np.empty(N, dtype=np.int64)
    for j in range(nc8):
        lo = j * NSL
        hi = min(N, lo + NSL)
        n = hi - lo
        order = np.argsort(-deg[lo:hi], kind="stable")
        rr = np.arange(n)
        rnd, idx = rr // TPS, rr % TPS
        tile_i = np.where(rnd % 2 == 0, idx, TPS - 1 - idx)
        pos = tile_i * 128 + rnd
        nl = np.empty(n, dtype=np.int64)
        nl[order] = pos
        newloc[lo:hi] = nl
    cfg.newloc = newloc

    er = np.minimum(row // NSL, nc8 - 1) * NLOC + newloc[row]  # new dest id
    ksrc = np.minimum(col // NSL, nc8 - 1)
    ecl = newloc[col]                                          # new src local

    # one global sort by (src core, dest id) replaces 8 masked argsorts
    o = np.argsort((ksrc * (nc8 * NLOC) + er).astype(np.int32),
                   kind="stable")
    ers, ecs, egs = er[o], ecl[o], rel[o]
    kofs = np.concatenate([[0], np.cumsum(np.bincount(ksrc, minlength=nc8))])
    counts = np.zeros((nc8, NT), dtype=np.int64)
    percore = []
    for k in range(nc8):
        sl = slice(kofs[k], kofs[k + 1])
        erk, eck, egk = ers[sl], ecs[sl], egs[sl]
        t = erk >> 7
        counts[k] = np.bincount(t, minlength=NT)
        percore.append((erk, eck, egk, t))

    chunks_t = np.maximum(1, np.ceil(counts.max(axis=0) / 128).astype(np.int64))
    # processing order = cfg.torder (half-major); slots follow that order
    torder = np.asarray(cfg.torder, dtype=np.int64)
    chunks_proc = chunks_t[torder]
    CH = int(chunks_proc.sum())
    CHpad = math.ceil(CH / B) * B
    NB = CHpad // B
    sb = np.concatenate([[0], np.cumsum(chunks_proc * 128)])[:-1]
    slot_base = np.empty(NT, dtype=np.int64)
    slot_base[torder] = sb

    x16 = x.astype(BF_NP)
    W1b = np.asarray(W1, dtype=np.float32).astype(BF_NP)
    W2cat = (np.asarray(W2, dtype=np.float32)
             .reshape(R, cfg.H, cfg.C).transpose(1, 0, 2)
             .reshape(cfg.H, R * cfg.C).astype(BF_NP))
    b1c = np.asarray(b1, dtype=np.float32).reshape(cfg.H, 1)
    b2r = np.broadcast_to(np.asarray(b2, dtype=np.float32),
                          (128, cfg.C)).copy()

    def core_inputs(k):
        erk, eck, egk, t = percore[k]
        first = np.searchsorted(t, np.arange(NT), side="left")
        rank = np.arange(len(t)) - first[t]
        slots = slot_base[t] + rank
        gidx = np.zeros(CHpad * 128, dtype=np.int16)
        gidx[slots] = (eck * R + egk).astype(np.int16)
        dloc = np.full(CHpad * 128, 200.0, dtype=np.float32)
        dloc[slots] = (erk % 128).astype(np.float32)

        # wrapped-16 index layout, compact (replicated to 128 on device):
        # slot i of batch b -> partition i%16, free column i//16
        gw = np.ascontiguousarray(
            gidx.reshape(NB, B * 8, 16).transpose(2, 0, 1)  # [16, NB, B*8]
        ).reshape(16, NB * B * 8)
        dloc_w = np.ascontiguousarray(
            dloc.reshape(CHpad, 128).T).astype(np.uint8)     # [128, CHpad]

        lo = k * NSL
        hi = min(N, lo + NSL)
        xk = np.zeros((NLOC, cfg.F), dtype=BF_NP)
        xk[newloc[lo:hi]] = x16[lo:hi]
        xTb = np.ascontiguousarray(xk.view(np.uint16).T).view(BF_NP)
        dk = np.zeros(NLOC, dtype=np.float32)
        dk[newloc[lo:hi]] = deg[lo:hi]
        degc = np.ascontiguousarray(dk.reshape(MC, 128).T)   # [128, MC]

        return {
            "xT": xTb,
            "degc": degc,
            "W1": W1b,
            "W2c": W2cat,
            "b1c": b1c,
            "b2r": b2r,
            "gidx": gw,
            "dloc": dloc_w,
        }

    from concurrent.futures import ThreadPoolExecutor
    with ThreadPoolExecutor(max_workers=nc8) as ex:
        in_maps = list(ex.map(core_inputs, range(nc8)))
    return in_maps, tuple(int(v) for v in chunks_proc), CHpad


def assemble(cfg, outs):
    """Un-permute per-core outputs into the full [N, C] array."""
    full = np.empty((cfg.N, cfg.C), dtype=np.float32)
    for j in range(cfg.ncores):
        lo = j * cfg.NSL
        hi = min(cfg.N, lo + cfg.NSL)
        full[lo:hi] = outs[j][cfg.newloc[lo:hi]].astype(np.float32)
    return full


# --------------------------------------------------------------- device side
def build_program(cfg, chunks_t, CHpad):
    R, H, C, F = cfg.R, cfg.H, cfg.C, cfg.F
    NB = CHpad // cfg.B
    nc = bacc.Bacc("TRN2", target_bir_lowering=False, debug=False,
                   num_devices=cfg.ncores)

    xT = nc.dram_tensor("xT", [F, cfg.NLOC], BF16, kind="ExternalInput").ap()
    degc = nc.dram_tensor("degc", [128, cfg.MC], F32, kind="ExternalInput").ap()
    W1 = nc.dram_tensor("W1", [R * F, H], BF16, kind="ExternalInput").ap()
    W2c = nc.dram_tensor("W2c", [H, R * C], BF16, kind="ExternalInput").ap()
    b1c = nc.dram_tensor("b1c", [H, 1], F32, kind="ExternalInput").ap()
    b2r = nc.dram_tensor("b2r", [128, C], F32, kind="ExternalInput").ap()
    gidx = nc.dram_tensor("gidx", [16, NB * cfg.B * 8], I16,
                          kind="ExternalInput").ap()
    dloc = nc.dram_tensor("dloc", [128, CHpad], mybir.dt.uint8,
                          kind="ExternalInput").ap()
    out = nc.dram_tensor("out", [cfg.NLOC, C], BF16,
                         kind="ExternalOutput").ap()

    with tile.TileContext(nc) as tc:
        _build(tc, cfg, chunks_t, CHpad, xT, degc, W1, W2c, b1c, b2r,
               gidx, dloc, out)
    nc.compile()
    return nc


def _build(tc, cfg, chunks_t, CHpad, xT, degc, W1, W2c, b1c, b2r,
           gidx, dloc, out):
    nc = tc.nc
    R, H, C = cfg.R, cfg.H, cfg.C
    B, J, MC, NT, KC = cfg.B, cfg.J, cfg.MC, cfg.NT, cfg.KC
    NB = CHpad // B
    RC = R * C
    B8 = B * 8
    with tc.tile_pool(name="const", bufs=1) as cpool, \
         tc.tile_pool(name="big", bufs=1) as bigp, \
         tc.tile_pool(name="gY", bufs=6) as gpool, \
         tc.tile_pool(name="s3", bufs=4) as spool, \
         tc.tile_pool(name="stage", bufs=6) as stpool, \
         tc.tile_pool(name="psum", bufs=6, space="PSUM") as pp, \
         tc.tile_pool(name="dram", bufs=1, space="DRAM") as dram:

        # ---------- constants
        iota16 = cpool.tile([128, 128], I16)
        nc.gpsimd.iota(iota16[:], pattern=[[1, 128]], base=0,
                       channel_multiplier=0)
        iotab = cpool.tile([128, 1, 128], BF16)
        nc.vector.tensor_copy(out=iotab[:, 0, :], in_=iota16[:])
        identf = cpool.tile([128, 128], F32)
        make_identity(nc, identf[:])
        identb = cpool.tile([128, 128], BF16)
        nc.vector.tensor_copy(out=identb[:], in_=identf[:])
        b2t = cpool.tile([128, C], F32)
        nc.sync.dma_start(out=b2t[:], in_=b2r[:, :])
        b1t = cpool.tile([H, 1], F32)
        nc.sync.dma_start(out=b1t[:], in_=b1c[:, :])
        b1tb = cpool.tile([H, 1], BF16)
        nc.vector.tensor_copy(out=b1tb[:], in_=b1t[:])
        w2t = cpool.tile([H, RC], BF16)
        nc.sync.dma_start(out=w2t[:], in_=W2c[:, :])
        w1t = cpool.tile([128, R * KC, H], BF16)
        nc.sync.dma_start(out=w1t[:],
                          in_=W1.rearrange("(q p) h -> p q h", p=128))

        degt = cpool.tile([128, MC], F32)
        nc.sync.dma_start(out=degt[:], in_=degc[:, :])
        dmask = cpool.tile([128, MC], F32)
        nc.vector.tensor_scalar(out=dmask[:], in0=degt[:], scalar1=0.0,
                                scalar2=None, op0=ALU.is_gt)
        dsq = cpool.tile([128, MC], F32)
        nc.scalar.sqrt(out=dsq[:], in_=degt[:])
        drcp = cpool.tile([128, MC], F32)
        nc.vector.reciprocal(out=drcp[:], in_=dsq[:])
        dinv = cpool.tile([128, MC], F32)
        nc.vector.tensor_mul(out=dinv[:], in0=drcp[:], in1=dmask[:])
        dinv2 = cpool.tile([128, MC], F32)
        nc.vector.tensor_mul(out=dinv2[:], in0=dinv[:], in1=dinv[:])

        # resident gather-index table, replicated 16 -> 128 partitions
        idxt = bigp.tile([128, NB * B8], I16)
        nc.sync.dma_start(out=idxt[0:16, :], in_=gidx[:, :])
        nc.sync.dma_start(out=idxt[16:32, :], in_=idxt[0:16, :])
        nc.sync.dma_start(out=idxt[32:64, :], in_=idxt[0:32, :])
        nc.sync.dma_start(out=idxt[64:128, :], in_=idxt[0:64, :])
        dloc8 = bigp.tile([128, CHpad], mybir.dt.uint8)
        nc.sync.dma_start(out=dloc8[:], in_=dloc[:, :])
        dloct = bigp.tile([128, CHpad, 1], BF16)
        nc.vector.tensor_copy(out=dloct[:, :, 0], in_=dloc8[:])

        # crow[r*C+c] = b1 @ W2_r, replicated to 128 partitions
        psc = pp.tile([1, RC], F32, tag="ps")
        nc.tensor.matmul(out=psc[:], lhsT=b1tb[:], rhs=w2t[:],
                         start=True, stop=True)
        crow1 = cpool.tile([1, RC], BF16)
        nc.scalar.copy(out=crow1[:], in_=psc[:])
        onesb = cpool.tile([1, 128], BF16)
        nc.vector.memset(onesb[:], 1.0)
        pscb = pp.tile([128, RC], F32, tag="ps")
        nc.tensor.matmul(out=pscb[:], lhsT=onesb[:], rhs=crow1[:],
                         start=True, stop=True)
        crow128 = cpool.tile([128, RC], F32)
        nc.scalar.copy(out=crow128[:], in_=pscb[:])

        # ---------- dense layer 1: y1[(m*R+r) row] = dinv[m]*(x[m] @ W1_r)
        uT = bigp.tile([128, KC, cfg.NLOC], BF16)
        nc.sync.dma_start(out=uT[:],
                          in_=xT.rearrange("(c p) n -> p c n", p=128))
        y1s = bigp.tile([128, MC, R, H], BF16)
        for mc in range(MC):
            for r in range(R):
                ps = pp.tile([128, H], F32, tag="ps")
                for kc in range(KC):
                    nc.tensor.matmul(
                        out=ps[:],
                        lhsT=uT[:, kc, mc * 128:(mc + 1) * 128],
                        rhs=w1t[:, r * KC + kc, :],
                        start=(kc == 0), stop=(kc == KC - 1))
                if (mc * R + r) % 2 == 0:
                    nc.scalar.mul(out=y1s[:, mc, r, :], in_=ps[:],
                                  mul=dinv[:, mc:mc + 1])
                else:
                    nc.vector.tensor_scalar(out=y1s[:, mc, r, :], in0=ps[:],
                                            scalar1=dinv[:, mc:mc + 1],
                                            scalar2=None, op0=ALU.mult)
        y1d = dram.tile([MC * 128 * R, H], BF16)
        nc.sync.dma_start(
            out=y1d.rearrange("(m p r) h -> p m r h", p=128, r=R),
            in_=y1s[:])

        LIMIT = int(os.environ.get("KLIMIT", "6"))
        if LIMIT < 2:
            return

        NTh = NT // 2
        MCh = MC // 2
        NL2 = cfg.NLOC // 2
        groups = [list(range(cfg.ncores))]

        def agg_pass(table_ap, width, partA, partB, GT, evac, mid=None):
            """Gather + one-hot matmul segment sum; staged group writes.

            Tiles are processed in the half-major order preprocess encoded in
            the slot layout; partA gets the first NT/2 tiles, partB the rest.
            `mid` is emitted between the halves (used to launch the first
            half's ReduceScatter so it overlaps the second half)."""
            pv = [partA.rearrange("(t p) w -> p t w", p=128),
                  partB.rearrange("(t p) w -> p t w", p=128)]
            c = 0
            s3 = None
            g = None
            stage = None
            for i in range(NT):
                half, pos = divmod(i, NTh)
                if pos % GT == 0:
                    stage = stpool.tile([128, GT, width], BF16,
                                        tag=f"stg{GT}_{width}")
                ps = pp.tile([128, width], F32, tag="ps")
                for j in range(chunks_t[i]):
                    if c % B == 0:
                        b = c // B
                        g = gpool.tile([128, B, 128], BF16, tag="g")
                        nc.gpsimd.dma_gather(
                            out_ap=g[:], in_ap=table_ap,
                            idxs_ap=idxt[:, b * B8:(b + 1) * B8],
                            num_idxs=B * 128, num_idxs_reg=B * 128,
                            elem_size=128)
                    if c % J == 0:
                        s3 = spool.tile([128, J, 128], BF16, tag="s3")
                        nj = min(J, CHpad - c)
                        nc.vector.tensor_tensor(
                            out=s3[:, :nj, :],
                            in0=dloct[:, c:c + nj, :].to_broadcast(
                                [128, nj, 128]),
                            in1=iotab[:].to_broadcast([128, nj, 128]),
                            op=ALU.is_equal)
                    nc.tensor.matmul(
                        out=ps[:], lhsT=s3[:, c % J, :],
                        rhs=g[:, c % B, :width],
                        start=(j == 0), stop=(j == chunks_t[i] - 1))
                    c += 1
                evac(stage[:, pos % GT, :], ps)
                if pos % GT == GT - 1:
                    t0 = pos - GT + 1
                    nc.sync.dma_start(out=pv[half][:, t0:t0 + GT, :],
                                      in_=stage[:])
                if pos == NTh - 1 and half == 0 and mid is not None:
                    mid()

        def evac_dve(dst, ps):
            nc.vector.tensor_copy(out=dst, in_=ps[:])

        def evac_act(dst, ps):
            nc.scalar.copy(out=dst, in_=ps[:])

        # ---------- layer-1 aggregation + split reduce-scatter (bf16)
        t1pA = dram.tile([NTh * 128, H], BF16)
        t1pB = dram.tile([NTh * 128, H], BF16)
        t1rA = dram.tile([NL2, H], BF16)
        t1rB = dram.tile([NL2, H], BF16)

        def rs1A():
            nc.gpsimd.collective_compute(
                "ReduceScatter", ALU.add, replica_groups=groups,
                ins=[t1pA.opt()], outs=[t1rA.opt()])

        agg_pass(y1d[:], H, t1pA, t1pB, cfg.GT1, evac_act, mid=rs1A)
        if LIMIT < 3:
            return
        nc.gpsimd.collective_compute(
            "ReduceScatter", ALU.add, replica_groups=groups,
            ins=[t1pB.opt()], outs=[t1rB.opt()])
        if LIMIT < 4:
            return

        # ---------- layer-2 dense: y2 rows (m*R+r), cols 0:C used
        u2T = bigp.tile([128, cfg.NLOC], BF16)
        y2s = bigp.tile([128, MC, RC], BF16)
        y2d = dram.tile([MC * 128 * R, 128], BF16)
        y2dv = y2d.rearrange("(m p r) h -> p m r h", p=128, r=R)
        for half, t1rh in ((0, t1rA), (1, t1rB)):
            m0 = half * MCh
            t1rs = bigp.tile([128, MCh, H], BF16, tag=f"t1rs{half}")
            nc.sync.dma_start(out=t1rs[:],
                              in_=t1rh.rearrange("(m p) h -> p m h", p=128))
            for mc in range(m0, m0 + MCh):
                tt = stpool.tile([128, H], BF16, tag="tt")
                nc.vector.tensor_scalar(out=tt[:], in0=t1rs[:, mc - m0, :],
                                        scalar1=dinv2[:, mc:mc + 1],
                                        scalar2=None, op0=ALU.mult)
                pst = pp.tile([128, 128], BF16, tag="ps")
                nc.tensor.transpose(out=pst[:], in_=tt[:], identity=identb[:])
                nc.vector.tensor_copy(out=u2T[:, mc * 128:(mc + 1) * 128],
                                      in_=pst[:])
            for mc in range(m0, m0 + MCh):
                ps2 = pp.tile([128, RC], F32, tag="ps")
                nc.tensor.matmul(out=ps2[:],
                                 lhsT=u2T[:, mc * 128:(mc + 1) * 128],
                                 rhs=w2t[:], start=True, stop=True)
                bias = stpool.tile([128, RC], F32, tag="bias")
                nc.vector.tensor_scalar(out=bias[:], in0=crow128[:],
                                        scalar1=dinv[:, mc:mc + 1],
                                        scalar2=None, op0=ALU.mult)
                nc.vector.tensor_tensor(out=y2s[:, mc, :], in0=ps2[:],
                                        in1=bias[:], op=ALU.add)
            for r in range(R):
                nc.sync.dma_start(
                    out=y2dv[:, m0:m0 + MCh, r, 0:C],
                    in_=y2s[:, m0:m0 + MCh, r * C:(r + 1) * C])
        if LIMIT < 5:
            return

        # ---------- layer-2 aggregation + split reduce-scatter
        t2pA = dram.tile([NTh * 128, C], BF16)
        t2pB = dram.tile([NTh * 128, C], BF16)
        t2rA = dram.tile([NL2, C], BF16)
        t2rB = dram.tile([NL2, C], BF16)

        def rs2A():
            nc.gpsimd.collective_compute(
                "ReduceScatter", ALU.add, replica_groups=groups,
                ins=[t2pA.opt()], outs=[t2rA.opt()])

        agg_pass(y2d[:], C, t2pA, t2pB, cfg.GT2, evac_act, mid=rs2A)
        nc.gpsimd.collective_compute(
            "ReduceScatter", ALU.add, replica_groups=groups,
            ins=[t2pB.opt()], outs=[t2rB.opt()])
        if LIMIT < 6:
            return

        # ---------- final: h2 = dinv*t2 + b2 ; fused log_softmax per half
        outv = out.rearrange("(m p) c -> p m c", p=128)
        for half, t2rh in ((0, t2rA), (1, t2rB)):
            m0 = half * MCh
            t2s = bigp.tile([128, MCh, C], BF16, tag=f"t2s{half}")
            nc.sync.dma_start(out=t2s[:],
                              in_=t2rh.rearrange("(m p) c -> p m c", p=128))
            ft = bigp.tile([128, MCh, C], F32, tag=f"ft{half}")
            nc.vector.tensor_tensor(
                out=ft[:], in0=t2s[:],
                in1=dinv[:, m0:m0 + MCh].unsqueeze(2).to_broadcast(
                    [128, MCh, C]), op=ALU.mult)
            nc.vector.tensor_tensor(
                out=ft[:], in0=ft[:],
                in1=b2t[:].unsqueeze(1).to_broadcast([128, MCh, C]),
                op=ALU.add)
            negmx = bigp.tile([128, MCh], F32, tag=f"mx{half}")
            nc.vector.tensor_reduce(out=negmx[:], in_=ft[:],
                                    axis=mybir.AxisListType.X,
                                    op=ALU.max, negate=True)
            nc.vector.tensor_tensor(
                out=ft[:], in0=ft[:],
                in1=negmx[:].unsqueeze(2).to_broadcast([128, MCh, C]),
                op=ALU.add)
            ex = bigp.tile([128, MCh, C], F32, tag=f"ex{half}")
            nc.scalar.activation(out=ex[:], in_=ft[:], func=AF.Exp)
            ssum = bigp.tile([128, MCh], F32, tag=f"sm{half}")
            nc.vector.tensor_reduce(out=ssum[:], in_=ex[:],
                                    axis=mybir.AxisListType.X, op=ALU.add)
            lg = bigp.tile([128, MCh], F32, tag=f"lg{half}")
            nc.scalar.activation(out=lg[:], in_=ssum[:], func=AF.Ln)
            fb = bigp.tile([128, MCh, C], BF16, tag=f"fb{half}")
            nc.vector.tensor_tensor(
                out=fb[:], in0=ft[:],
                in1=lg[:].unsqueeze(2).to_broadcast([128, MCh, C]),
                op=ALU.subtract)
            nc.sync.dma_start(out=outv[:, m0:m0 + MCh, :], in_=fb[:])


# ------------------------------------------------------------------ runtime
_PROGRAM_CACHE = {}


def run(cfg, inputs):
    in_maps, chunks_t, CHpad = preprocess(cfg, **inputs)
    key = (cfg.N, cfg.E, chunks_t, CHpad)
    if key not in _PROGRAM_CACHE:
        _PROGRAM_CACHE[key] = build_program(cfg, chunks_t, CHpad)
    nc = _PROGRAM_CACHE[key]
    res = None
    for attempt in range(3):
        try:
            res = run_bass_kernel_spmd(nc, in_maps,
                                       core_ids=list(range(cfg.ncores)))
            break
        except Exception:
            if attempt == 2:
                raise
    outs = [np.asarray(res.results[k]["out"]) for k in range(cfg.ncores)]
    return np.ascontiguousarray(assemble(cfg, outs).astype(np.float32))


def kernel(x, edge_index, edge_relation, W1, b1, W2, b2):
    return run(CFG, dict(x=x, edge_index=edge_index,
                         edge_relation=edge_relation,
                         W1=W1, b1=b1, W2=W2, b2=b2))
